# revision 29
# baseline (speedup 1.0000x reference)
"""AdaAttention (gumbel-gated sparse attention block) on 8 TRN2 NeuronCores.

Strategy: pure data-parallel over batch (64 batches -> 8 per core).  Each
core runs the full attention block for its 8 batches; no collectives.

Per-core layout (all f32):
  xt   [768, 1576]  x^T for this core's 8 batches (host transposes)
  wall [768, 2305]  concat([Wq, Wk, Wv, Wm], 0).T  (weights as lhsT tiles)
  wpt  [768, 768]   Wp.T
  bpt  [6, 128]     bp reshaped per 128-row output tile
  thr  [1, 1576]    gating threshold per token: 5*ln(1.5) - g1 + g2
                    (CLS slots = -1e30 so the CLS gate always passes)
  outt [768, 1576]  out^T (host transposes back)

Device pipeline:
  1. logits = Wm @ x^T (PE), ts = (logits > thr) via DVE is_gt; ts is
     round-tripped through DRAM to get per-partition column layout.
  2. QKV GEMM, transposed outputs: Q^T (pre-scaled by 1/8), K^T (spilled
     to DRAM, re-streamed per head-pair), V^T (doubles as the attention
     output buffer attnT: gated tokens pass v[n] through unchanged).
  3. V in normal layout ([token, head*65] with a ones column appended)
     for the PV matmul / softmax denominator.
  4. Per (batch, head): S^T = K_h @ Q_h^T (PE, odd/even heads land on
     row-groups 0/64 for concurrency); fused mask+exp in ONE ScalarE
     pass: exp(S*ts_m + 30*ts_m - 30) with per-partition scale/bias;
     PV^T+Z via [V|1] stationary; Z broadcast via a 1-row ones matmul;
     divide (DVE); copy_predicated overwrites kept-token columns of
     attnT (gated columns keep V^T).
  5. proj GEMM + bias, DMA out.
"""
import sys
import types

import numpy as np

# ---------------------------------------------------------------- patches
# This container's walrus rejects >1 sync-wait per instruction; Tile's
# kernel-tail drain aggregates one wait per outstanding proc.  Spread the
# waits across one sync-NOP each.  Also: the image's antenv lacks
# axon_hooks (NTFF profiling) and upload_artifacts wants a bucket.


def _install_patches():
    import bass_rust
    import concourse.tile as tile

    def _patched_drain_and_barrier(self, tick_clock, wait_clock):
        gc = tick_clock.global_clock
        ticks = eval(repr(gc).replace("VectorClock(", "").rstrip(")"))
        for i, t in enumerate(ticks):
            if t > 0:
                cur = list(ticks)
                cur[i] = 0
                nop = self.nc.sync.nop()
                wait_clock.add_sem_waits(
                    nop.ins,
                    tile.ScopedClock({None: gc}),
                    tile.ScopedClock({None: bass_rust.VectorClock(cur)}),
                )
        drain_inst = self.nc.sync.drain()
        wait_clock.add_sem_waits(
            drain_inst.ins, tile.ScopedClock({None: gc}), tile.ScopedClock({None: gc})
        )
        self.nc.all_engine_barrier()
        assert self.sems is not None
        popped = self.nc._tile_sem_poison_stack.pop()
        assert popped is self._sem_poison
        self.nc.clear_and_free_semaphores(list(self.sems.allocated().values()))
        self.nc.all_engine_barrier()

    tile.TileContext._drain_and_barrier = _patched_drain_and_barrier

    if "antenv.axon_hooks" not in sys.modules:
        mod = types.ModuleType("antenv.axon_hooks")
        try:
            from trn_agent_boot.trn_boot import _ntff_profile_via_ctypes

            hook = _ntff_profile_via_ctypes("/opt/axon/libaxon_pjrt.so")
        except Exception:
            hook = None
        mod.get_axon_ntff_profile_hook = lambda: hook
        mod.set_axon_ntff_profile_hook = lambda h: None
        sys.modules["antenv.axon_hooks"] = mod

    import concourse.bass_utils as bass_utils

    bass_utils.upload_artifacts = lambda tmpdir: f"file://{tmpdir}"


_install_patches()

import concourse.bass as bass  # noqa: E402
import concourse.mybir as mybir  # noqa: E402
import concourse.tile as tile  # noqa: E402
from concourse.bass_utils import run_bass_kernel_spmd  # noqa: E402


def _split_multi_waits(nc):
    """walrus here allows one sync-wait per engine instruction: hoist extra
    waits onto same-engine NoOps inserted immediately before."""
    for fn in nc.m.functions:
        for bb in fn.blocks:
            out = []
            changed = False
            for inst in bb.instructions:
                si = inst.sync_info
                waits = list(si.on_wait) if si is not None else []
                if len(waits) > 1:
                    changed = True
                    for k, w in enumerate(waits[:-1]):
                        nop = mybir.InstNoOp(
                            name=f"{inst.name}-w{k}",
                            engine=inst.engine,
                            ins=[],
                            outs=[],
                            sync_info=mybir.SyncInfo(on_wait=[w], on_update=[]),
                        )
                        out.append(nop)
                    si.on_wait = [waits[-1]]
                out.append(inst)
            if changed:
                try:
                    bb.instructions = out
                except Exception:
                    bb.instructions.clear()
                    bb.instructions.extend(out)

F32 = mybir.dt.float32
BF16 = mybir.dt.bfloat16
AF = mybir.ActivationFunctionType
OP = mybir.AluOpType

B, N, C = 64, 197, 768
H, D = 12, 64
NCORES = 8
BL = B // NCORES  # 8 batches per core
T = BL * N  # 1576 tokens per core
TCH = 394  # token chunk (= 2 batches); 4 chunks per core
KPAD = 40  # gathered (kept) tokens per batch, padded; data max is ~38
XG = BL * KPAD  # 512 gathered columns per core
NEG = 30.0
TP = 1584  # T padded to 16B-aligned fp8 stride
FP8 = mybir.dt.float8e4
SW, SF = 16.0, 256.0  # host-side weight scales for fp8 range


def build_nc():
    nc = bass.Bass()
    xb_d = nc.declare_dram_parameter("xb", [C, T], BF16, isOutput=False)
    xg_d = nc.declare_dram_parameter("xg", [C, XG], BF16, isOutput=False)
    wqk_d = nc.declare_dram_parameter("wqk", [C, 2 * C], BF16, isOutput=False)
    wvt_d = nc.declare_dram_parameter("wvt", [C, C], BF16, isOutput=False)
    wf_d = nc.declare_dram_parameter("wf", [C, C], BF16, isOutput=False)
    wpt_d = nc.declare_dram_parameter("wpt", [C, C], BF16, isOutput=False)
    bpt_d = nc.declare_dram_parameter("bpt", [6, 128], F32, isOutput=False)
    bgm_d = nc.declare_dram_parameter("bgm", [BL, KPAD], F32, isOutput=False)
    out_d = nc.declare_dram_parameter("outt", [C, T], F32, isOutput=True)
    outs_d = nc.declare_dram_parameter("outs", [C, XG], F32, isOutput=True)

    rzd = nc.dram_tensor("rzd", [24, 4, KPAD], F32, kind="Internal")

    with tile.TileContext(nc) as tc:
        from contextlib import ExitStack

        with ExitStack() as ctx:
            E = ctx.enter_context
            xpool = E(tc.tile_pool(name="xb", bufs=6))
            xgpool = E(tc.tile_pool(name="xg", bufs=6))
            qgpool = E(tc.tile_pool(name="qg", bufs=12))
            vgpool = E(tc.tile_pool(name="vg", bufs=8))
            agpool = E(tc.tile_pool(name="atg", bufs=6))
            w1pool = E(tc.tile_pool(name="w128", bufs=12))
            w3pool = E(tc.tile_pool(name="w384", bufs=14))
            ppool = E(tc.tile_pool(name="pp", bufs=16))
            rzpool = E(tc.tile_pool(name="rz", bufs=4))
            zbpool = E(tc.tile_pool(name="zbb", bufs=8))
            opool = E(tc.tile_pool(name="out", bufs=4))
            scpool = E(tc.tile_pool(name="smallcol", bufs=24))
            ps_mm = E(tc.tile_pool(name="ps_mm", bufs=2, space="PSUM"))
            ps_s = E(tc.tile_pool(name="ps_s", bufs=2, space="PSUM"))
            ps_pv = E(tc.tile_pool(name="ps_pv", bufs=2, space="PSUM"))
            ps_z = E(tc.tile_pool(name="ps_z", bufs=2, space="PSUM"))

            # ---- constants / inputs
            ones_bf = scpool.tile([128, 1], BF16, tag="onesb")
            nc.vector.memset(ones_bf[:], 1.0)

            xg = []
            for c in range(6):
                g_ = xgpool.tile([128, XG], BF16, name="xg", tag="xg")
                nc.sync.dma_start(g_[:], xg_d[c * 128 : (c + 1) * 128, :])
                xg.append(g_)
            wqk_sb, wvt_sb, wf_sb = [], [], []
            for c in range(6):
                t_ = w1pool.tile([128, 2 * C], BF16, name="wqk", tag="wqk", bufs=6)
                nc.sync.dma_start(t_[:], wqk_d[c * 128 : (c + 1) * 128, :])
                wqk_sb.append(t_)
            for c in range(6):
                t_ = w3pool.tile([128, C], BF16, name="wvt", tag="wvt", bufs=6)
                nc.sync.dma_start(t_[:], wvt_d[c * 128 : (c + 1) * 128, :])
                wvt_sb.append(t_)
            xb = []
            for c in range(6):
                t_ = xpool.tile([128, T], BF16, name="xb", tag="xb")
                nc.sync.dma_start(t_[:], xb_d[c * 128 : (c + 1) * 128, :])
                xb.append(t_)
            for c in range(6):
                t_ = w1pool.tile([128, C], BF16, name="wf", tag="wf", bufs=6)
                nc.sync.dma_start(t_[:], wf_d[c * 128 : (c + 1) * 128, :])
                wf_sb.append(t_)
            wpt_sb = []
            for f in range(6):
                t_ = w1pool.tile([128, C], BF16, name="wpt", tag="wpt", bufs=6)
                nc.sync.dma_start(t_[:], wpt_d[f * 128 : (f + 1) * 128, :])
                wpt_sb.append(t_)

            bp_sb = []
            for ct in range(6):
                t_ = scpool.tile([128, 1], F32, tag="bp")
                nc.sync.dma_start(
                    t_[:], bpt_d[ct : ct + 1, :].rearrange("a b -> b a")
                )
                bp_sb.append(t_)

            biasg = []
            for b in range(BL):
                t_ = scpool.tile([128, 1], F32, tag="bg")
                nc.sync.dma_start(
                    t_[0:KPAD, :], bgm_d[b : b + 1, :].rearrange("a b -> b a")
                )
                biasg.append(t_)

            # ---- GEMM B: Q^T,K^T on gathered tokens (Q pre-scaled by 1/8)
            qg, kg = [], []
            for f in range(12):
                pm = ps_mm.tile([128, XG], F32, tag="mm")
                for c in range(6):
                    nc.tensor.matmul(
                        pm[:], wqk_sb[c][:, f * 128 : (f + 1) * 128], xg[c][:],
                        start=(c == 0), stop=(c == 5),
                    )
                dst = qgpool.tile([128, XG], BF16, name="qkg", tag="qkg")
                if f < 6:
                    nc.vector.tensor_scalar_mul(dst[:], pm[:], 0.125)
                    qg.append(dst)
                else:
                    nc.scalar.copy(dst[:], pm[:])
                    kg.append(dst)

            # ---- GEMM C: V on gathered tokens, token-partition layout
            vg = []
            for b in range(BL):
                vg.append(vgpool.tile([KPAD, C], BF16, name="vg", tag="vg"))
            for n2 in range(2):
                for bp2 in range(4):
                    b0, b1 = 2 * bp2, 2 * bp2 + 1
                    pm = ps_mm.tile([128, 384], F32, tag="mm")
                    for c in range(6):
                        wsl = wvt_sb[c][:, n2 * 384 : (n2 + 1) * 384]
                        nc.tensor.matmul(
                            pm[0:KPAD, :],
                            xg[c][:, b0 * KPAD : (b0 + 1) * KPAD],
                            wsl,
                            start=(c == 0),
                            stop=(c == 5),
                        )
                        nc.tensor.matmul(
                            pm[64 : 64 + KPAD, :],
                            xg[c][:, b1 * KPAD : (b1 + 1) * KPAD],
                            wsl,
                            start=(c == 0),
                            stop=(c == 5),
                        )
                    nc.scalar.copy(
                        vg[b0][:, n2 * 384 : (n2 + 1) * 384], pm[0:KPAD, :]
                    )
                    nc.vector.tensor_copy(
                        vg[b1][:, n2 * 384 : (n2 + 1) * 384], pm[64 : 64 + KPAD, :]
                    )

            atg = []
            for f in range(6):
                atg.append(agpool.tile([128, XG], BF16, name="atg", tag="atg"))

            # interleave plan for GEMM A (dense pass-through): 3 psum-groups
            # of A per attention batch
            a_groups = [(ct, tch) for ct in range(6) for tch in range(4)]

            def emit_a_group(ct, tch):
                sl = slice(tch * TCH, (tch + 1) * TCH)
                pm = ps_mm.tile([128, TCH], F32, tag="mm")
                for c in range(6):
                    nc.tensor.matmul(
                        pm[:], wf_sb[c][:, ct * 128 : (ct + 1) * 128], xb[c][:, sl],
                        start=(c == 0), stop=(c == 5),
                    )
                ot = opool.tile([128, TCH], F32, name="ot", tag="ot")
                nc.scalar.activation(ot[:], pm[:], AF.Identity, bias=bp_sb[ct][:])
                nc.sync.dma_start(out_d[ct * 128 : (ct + 1) * 128, sl], ot[:])

            # ---- attention on gathered tokens, batch-major
            for b in range(BL):
                gsl = slice(b * KPAD, (b + 1) * KPAD)
                for q in range(3):
                    pps = {}
                    zps = ps_z.tile([97, KPAD], F32, tag="z")
                    pvs = []
                    for k in range(4):
                        h = 4 * q + k
                        hp, hb = h // 2, (h % 2) * 64
                        ss = ps_s.tile([KPAD, KPAD], F32, tag="s")
                        nc.tensor.matmul(
                            ss[:], kg[hp][hb : hb + 64, gsl], qg[hp][hb : hb + 64, gsl]
                        )
                        pp = ppool.tile([KPAD, KPAD], BF16, tag="pp")
                        nc.scalar.activation(
                            pp[:], ss[:], AF.Exp, bias=biasg[b][0:KPAD, :]
                        )
                        pps[k] = pp
                    for pair in range(2):
                        pv2 = ps_pv.tile([128, KPAD], F32, tag="pv")
                        pvs.append(pv2)
                        for hh in range(2):
                            k = 2 * pair + hh
                            h = 4 * q + k
                            nc.tensor.matmul(
                                pv2[64 * hh : 64 * hh + 64, :],
                                vg[b][0:KPAD, h * 64 : (h + 1) * 64],
                                pps[k][:],
                            )
                            nc.tensor.matmul(
                                zps[32 * k : 32 * k + 1, :],
                                ones_bf[0:KPAD, :],
                                pps[k][:],
                                tile_position=(0, 32 * k),
                            )
                    rzq = rzpool.tile([97, KPAD], F32, tag="rz")
                    nc.vector.reciprocal(rzq[:], zps[:])
                    qq = b * 3 + q
                    nc.gpsimd.dma_start(rzd[qq, :, :], rzq[0 : 97 : 32, :])
                    for pair in range(2):
                        h0 = 4 * q + 2 * pair
                        hp = h0 // 2
                        zbb = zbpool.tile([128, KPAD], F32, tag="zbb")
                        for hh in range(2):
                            k = 2 * pair + hh
                            nc.gpsimd.dma_start(
                                zbb[64 * hh : 64 * hh + 64, :],
                                rzd[qq, k : k + 1, :].to_broadcast((64, KPAD)),
                            )
                        nc.vector.tensor_tensor(
                            atg[hp][:, gsl], pvs[pair][:], zbb[:], op=OP.mult
                        )
                # 3 dense-pass groups between batches keep the PE fed
                for gi in range(3):
                    idx = b * 3 + gi
                    if idx < len(a_groups):
                        emit_a_group(*a_groups[idx])

            for ct, tch in a_groups[24:]:
                emit_a_group(ct, tch)

            # ---- GEMM D: proj of the gathered (kept) columns
            for ct in range(6):
                pm = ps_mm.tile([128, XG], F32, tag="mm")
                for f in range(6):
                    nc.tensor.matmul(
                        pm[:], wpt_sb[f][:, ct * 128 : (ct + 1) * 128], atg[f][:],
                        start=(f == 0), stop=(f == 5),
                    )
                ot = opool.tile([128, XG], F32, name="ots", tag="ot")
                nc.scalar.activation(ot[:], pm[:], AF.Identity, bias=bp_sb[ct][:])
                nc.sync.dma_start(outs_d[ct * 128 : (ct + 1) * 128, :], ot[:])

    _split_multi_waits(nc)
    return nc


_NC = None


def _get_nc():
    global _NC
    if _NC is None:
        _NC = build_nc()
    return _NC


def _host_gate(x, g1, g2, Wm, bm):
    """ts gate on the host (tiny GEMV), returns kept index lists per batch."""
    logits = (
        x[:, 1:].reshape(-1, C).astype(np.float32) @ np.asarray(Wm, np.float32).T
    ).reshape(B, N - 1) + np.asarray(bm, np.float32)[0]
    thr = 5.0 * np.log(1.5) - np.asarray(g1, np.float32)[..., 0] + np.asarray(
        g2, np.float32
    )[..., 0]
    keep = logits > thr  # (B, N-1)
    idx = []
    for b in range(B):
        ii = [0] + [int(i) + 1 for i in np.nonzero(keep[b])[0]]
        assert len(ii) <= KPAD, f"batch {b}: {len(ii)} kept tokens > KPAD={KPAD}"
        idx.append(ii)
    return idx


def make_in_maps(x, g1, g2, Wq, Wk, Wv, Wp, bp, Wm, bm):
    import ml_dtypes

    bf16 = ml_dtypes.bfloat16
    x = np.asarray(x, np.float32)
    idx = _host_gate(x, g1, g2, Wm, bm)
    Wq, Wk, Wv, Wp = (np.asarray(w, np.float32) for w in (Wq, Wk, Wv, Wp))
    wqk = np.ascontiguousarray(np.concatenate([Wq, Wk], 0).T.astype(bf16))
    wvt = np.ascontiguousarray(Wv.T.astype(bf16))
    wf = np.ascontiguousarray((Wp @ Wv).T.astype(bf16))
    wpt = np.ascontiguousarray(Wp.T.astype(bf16))
    bpt = np.ascontiguousarray(np.asarray(bp, np.float32).reshape(6, 128))
    in_maps = []
    for i in range(NCORES):
        xs = x[i * BL : (i + 1) * BL]  # (BL, N, C)
        xtp = xs.reshape(T, C).T  # (C, T)
        cols = np.zeros(XG, np.int64)
        bgm = np.full((BL, KPAD), -NEG, np.float32)
        for bl in range(BL):
            ii = idx[i * BL + bl]
            kb = len(ii)
            cols[bl * KPAD : bl * KPAD + kb] = bl * N + np.asarray(ii)
            bgm[bl, :kb] = 0.0
        xgp = np.ascontiguousarray(xtp[:, cols].astype(bf16))
        in_maps.append(
            {
                "xb": np.ascontiguousarray(xtp.astype(bf16)),
                "xg": xgp,
                "wqk": wqk,
                "wvt": wvt,
                "wf": wf,
                "wpt": wpt,
                "bpt": bpt,
                "bgm": bgm,
            }
        )
    return in_maps, idx


def run(in_maps, trace=False, tmpdir=None):
    nc = _get_nc()
    return run_bass_kernel_spmd(
        nc, in_maps, core_ids=list(range(NCORES)), trace=trace, tmpdir=tmpdir
    )


def merge_outputs(res, idx):
    out = np.empty((B, N, C), np.float32)
    for i in range(NCORES):
        dense = res.results[i]["outt"].T.reshape(BL, N, C)
        sparse = res.results[i]["outs"].T  # (XG, C)
        out[i * BL : (i + 1) * BL] = dense
        for bl in range(BL):
            ii = idx[i * BL + bl]
            out[i * BL + bl, ii, :] = sparse[bl * KPAD : bl * KPAD + len(ii)]
    return out


def kernel(**inputs):
    in_maps, idx = make_in_maps(**inputs)
    res = run(in_maps)
    return merge_outputs(res, idx)


# revision 30
# speedup vs baseline: 1.0186x; 1.0186x over previous
"""AdaAttention (gumbel-gated sparse attention block) on 8 TRN2 NeuronCores.

Strategy: pure data-parallel over batch (64 batches -> 8 per core).  Each
core runs the full attention block for its 8 batches; no collectives.

Per-core layout (all f32):
  xt   [768, 1576]  x^T for this core's 8 batches (host transposes)
  wall [768, 2305]  concat([Wq, Wk, Wv, Wm], 0).T  (weights as lhsT tiles)
  wpt  [768, 768]   Wp.T
  bpt  [6, 128]     bp reshaped per 128-row output tile
  thr  [1, 1576]    gating threshold per token: 5*ln(1.5) - g1 + g2
                    (CLS slots = -1e30 so the CLS gate always passes)
  outt [768, 1576]  out^T (host transposes back)

Device pipeline:
  1. logits = Wm @ x^T (PE), ts = (logits > thr) via DVE is_gt; ts is
     round-tripped through DRAM to get per-partition column layout.
  2. QKV GEMM, transposed outputs: Q^T (pre-scaled by 1/8), K^T (spilled
     to DRAM, re-streamed per head-pair), V^T (doubles as the attention
     output buffer attnT: gated tokens pass v[n] through unchanged).
  3. V in normal layout ([token, head*65] with a ones column appended)
     for the PV matmul / softmax denominator.
  4. Per (batch, head): S^T = K_h @ Q_h^T (PE, odd/even heads land on
     row-groups 0/64 for concurrency); fused mask+exp in ONE ScalarE
     pass: exp(S*ts_m + 30*ts_m - 30) with per-partition scale/bias;
     PV^T+Z via [V|1] stationary; Z broadcast via a 1-row ones matmul;
     divide (DVE); copy_predicated overwrites kept-token columns of
     attnT (gated columns keep V^T).
  5. proj GEMM + bias, DMA out.
"""
import sys
import types

import numpy as np

# ---------------------------------------------------------------- patches
# This container's walrus rejects >1 sync-wait per instruction; Tile's
# kernel-tail drain aggregates one wait per outstanding proc.  Spread the
# waits across one sync-NOP each.  Also: the image's antenv lacks
# axon_hooks (NTFF profiling) and upload_artifacts wants a bucket.


def _install_patches():
    import bass_rust
    import concourse.tile as tile

    def _patched_drain_and_barrier(self, tick_clock, wait_clock):
        gc = tick_clock.global_clock
        ticks = eval(repr(gc).replace("VectorClock(", "").rstrip(")"))
        for i, t in enumerate(ticks):
            if t > 0:
                cur = list(ticks)
                cur[i] = 0
                nop = self.nc.sync.nop()
                wait_clock.add_sem_waits(
                    nop.ins,
                    tile.ScopedClock({None: gc}),
                    tile.ScopedClock({None: bass_rust.VectorClock(cur)}),
                )
        drain_inst = self.nc.sync.drain()
        wait_clock.add_sem_waits(
            drain_inst.ins, tile.ScopedClock({None: gc}), tile.ScopedClock({None: gc})
        )
        self.nc.all_engine_barrier()
        assert self.sems is not None
        popped = self.nc._tile_sem_poison_stack.pop()
        assert popped is self._sem_poison
        self.nc.clear_and_free_semaphores(list(self.sems.allocated().values()))

    tile.TileContext._drain_and_barrier = _patched_drain_and_barrier

    if "antenv.axon_hooks" not in sys.modules:
        mod = types.ModuleType("antenv.axon_hooks")
        try:
            from trn_agent_boot.trn_boot import _ntff_profile_via_ctypes

            hook = _ntff_profile_via_ctypes("/opt/axon/libaxon_pjrt.so")
        except Exception:
            hook = None
        mod.get_axon_ntff_profile_hook = lambda: hook
        mod.set_axon_ntff_profile_hook = lambda h: None
        sys.modules["antenv.axon_hooks"] = mod

    import concourse.bass_utils as bass_utils

    bass_utils.upload_artifacts = lambda tmpdir: f"file://{tmpdir}"


_install_patches()

import concourse.bass as bass  # noqa: E402
import concourse.mybir as mybir  # noqa: E402
import concourse.tile as tile  # noqa: E402
from concourse.bass_utils import run_bass_kernel_spmd  # noqa: E402


def _split_multi_waits(nc):
    """walrus here allows one sync-wait per engine instruction: hoist extra
    waits onto same-engine NoOps inserted immediately before."""
    for fn in nc.m.functions:
        for bb in fn.blocks:
            out = []
            changed = False
            for inst in bb.instructions:
                si = inst.sync_info
                waits = list(si.on_wait) if si is not None else []
                if len(waits) > 1:
                    changed = True
                    for k, w in enumerate(waits[:-1]):
                        nop = mybir.InstNoOp(
                            name=f"{inst.name}-w{k}",
                            engine=inst.engine,
                            ins=[],
                            outs=[],
                            sync_info=mybir.SyncInfo(on_wait=[w], on_update=[]),
                        )
                        out.append(nop)
                    si.on_wait = [waits[-1]]
                out.append(inst)
            if changed:
                try:
                    bb.instructions = out
                except Exception:
                    bb.instructions.clear()
                    bb.instructions.extend(out)

F32 = mybir.dt.float32
BF16 = mybir.dt.bfloat16
AF = mybir.ActivationFunctionType
OP = mybir.AluOpType

B, N, C = 64, 197, 768
H, D = 12, 64
NCORES = 8
BL = B // NCORES  # 8 batches per core
T = BL * N  # 1576 tokens per core
TCH = 394  # token chunk (= 2 batches); 4 chunks per core
KPAD = 40  # gathered (kept) tokens per batch, padded; data max is ~38
XG = BL * KPAD  # 512 gathered columns per core
NEG = 30.0
TP = 1584  # T padded to 16B-aligned fp8 stride
FP8 = mybir.dt.float8e4
SW, SF = 16.0, 256.0  # host-side weight scales for fp8 range


def build_nc():
    nc = bass.Bass()
    xb_d = nc.declare_dram_parameter("xb", [C, T], BF16, isOutput=False)
    xg_d = nc.declare_dram_parameter("xg", [C, XG], BF16, isOutput=False)
    wqk_d = nc.declare_dram_parameter("wqk", [C, 2 * C], BF16, isOutput=False)
    wvt_d = nc.declare_dram_parameter("wvt", [C, C], BF16, isOutput=False)
    wf_d = nc.declare_dram_parameter("wf", [C, C], BF16, isOutput=False)
    wpt_d = nc.declare_dram_parameter("wpt", [C, C], BF16, isOutput=False)
    bpt_d = nc.declare_dram_parameter("bpt", [6, 128], F32, isOutput=False)
    bgm_d = nc.declare_dram_parameter("bgm", [BL, KPAD], F32, isOutput=False)
    out_d = nc.declare_dram_parameter("outt", [C, T], F32, isOutput=True)
    outs_d = nc.declare_dram_parameter("outs", [C, XG], F32, isOutput=True)

    rzd = nc.dram_tensor("rzd", [24, 4, KPAD], F32, kind="Internal")

    with tile.TileContext(nc) as tc:
        from contextlib import ExitStack

        with ExitStack() as ctx:
            E = ctx.enter_context
            xpool = E(tc.tile_pool(name="xb", bufs=6))
            xgpool = E(tc.tile_pool(name="xg", bufs=6))
            qgpool = E(tc.tile_pool(name="qg", bufs=12))
            vgpool = E(tc.tile_pool(name="vg", bufs=8))
            agpool = E(tc.tile_pool(name="atg", bufs=6))
            w1pool = E(tc.tile_pool(name="w128", bufs=12))
            w3pool = E(tc.tile_pool(name="w384", bufs=14))
            ppool = E(tc.tile_pool(name="pp", bufs=16))
            rzpool = E(tc.tile_pool(name="rz", bufs=4))
            zbpool = E(tc.tile_pool(name="zbb", bufs=8))
            opool = E(tc.tile_pool(name="out", bufs=4))
            scpool = E(tc.tile_pool(name="smallcol", bufs=24))
            ps_mm = E(tc.tile_pool(name="ps_mm", bufs=2, space="PSUM"))
            ps_s = E(tc.tile_pool(name="ps_s", bufs=2, space="PSUM"))
            ps_pv = E(tc.tile_pool(name="ps_pv", bufs=2, space="PSUM"))
            ps_z = E(tc.tile_pool(name="ps_z", bufs=2, space="PSUM"))

            # ---- constants / inputs
            ones_bf = scpool.tile([128, 1], BF16, tag="onesb")
            nc.vector.memset(ones_bf[:], 1.0)

            xg = []
            for c in range(6):
                g_ = xgpool.tile([128, XG], BF16, name="xg", tag="xg")
                nc.sync.dma_start(g_[:], xg_d[c * 128 : (c + 1) * 128, :])
                xg.append(g_)
            wqk_sb, wvt_sb, wf_sb = [], [], []
            for c in range(6):
                t_ = w1pool.tile([128, 2 * C], BF16, name="wqk", tag="wqk", bufs=6)
                nc.sync.dma_start(t_[:], wqk_d[c * 128 : (c + 1) * 128, :])
                wqk_sb.append(t_)
            for c in range(6):
                t_ = w3pool.tile([128, C], BF16, name="wvt", tag="wvt", bufs=6)
                nc.sync.dma_start(t_[:], wvt_d[c * 128 : (c + 1) * 128, :])
                wvt_sb.append(t_)
            xb = []
            for c in range(6):
                t_ = xpool.tile([128, T], BF16, name="xb", tag="xb")
                nc.sync.dma_start(t_[:], xb_d[c * 128 : (c + 1) * 128, :])
                xb.append(t_)
            for c in range(6):
                t_ = w1pool.tile([128, C], BF16, name="wf", tag="wf", bufs=6)
                nc.sync.dma_start(t_[:], wf_d[c * 128 : (c + 1) * 128, :])
                wf_sb.append(t_)
            wpt_sb = []
            for f in range(6):
                t_ = w1pool.tile([128, C], BF16, name="wpt", tag="wpt", bufs=6)
                nc.sync.dma_start(t_[:], wpt_d[f * 128 : (f + 1) * 128, :])
                wpt_sb.append(t_)

            bp_sb = []
            for ct in range(6):
                t_ = scpool.tile([128, 1], F32, tag="bp")
                nc.sync.dma_start(
                    t_[:], bpt_d[ct : ct + 1, :].rearrange("a b -> b a")
                )
                bp_sb.append(t_)

            biasg = []
            for b in range(BL):
                t_ = scpool.tile([128, 1], F32, tag="bg")
                nc.sync.dma_start(
                    t_[0:KPAD, :], bgm_d[b : b + 1, :].rearrange("a b -> b a")
                )
                biasg.append(t_)

            # ---- GEMM B: Q^T,K^T on gathered tokens (Q pre-scaled by 1/8)
            qg, kg = [], []
            for f in range(12):
                pm = ps_mm.tile([128, XG], F32, tag="mm")
                for c in range(6):
                    nc.tensor.matmul(
                        pm[:], wqk_sb[c][:, f * 128 : (f + 1) * 128], xg[c][:],
                        start=(c == 0), stop=(c == 5),
                    )
                dst = qgpool.tile([128, XG], BF16, name="qkg", tag="qkg")
                if f < 6:
                    nc.vector.tensor_scalar_mul(dst[:], pm[:], 0.125)
                    qg.append(dst)
                else:
                    nc.scalar.copy(dst[:], pm[:])
                    kg.append(dst)

            # ---- GEMM C: V on gathered tokens, token-partition layout
            vg = []
            for b in range(BL):
                vg.append(vgpool.tile([KPAD, C], BF16, name="vg", tag="vg"))
            for n2 in range(2):
                for bp2 in range(4):
                    b0, b1 = 2 * bp2, 2 * bp2 + 1
                    pm = ps_mm.tile([128, 384], F32, tag="mm")
                    for c in range(6):
                        wsl = wvt_sb[c][:, n2 * 384 : (n2 + 1) * 384]
                        nc.tensor.matmul(
                            pm[0:KPAD, :],
                            xg[c][:, b0 * KPAD : (b0 + 1) * KPAD],
                            wsl,
                            start=(c == 0),
                            stop=(c == 5),
                        )
                        nc.tensor.matmul(
                            pm[64 : 64 + KPAD, :],
                            xg[c][:, b1 * KPAD : (b1 + 1) * KPAD],
                            wsl,
                            start=(c == 0),
                            stop=(c == 5),
                        )
                    nc.scalar.copy(
                        vg[b0][:, n2 * 384 : (n2 + 1) * 384], pm[0:KPAD, :]
                    )
                    nc.vector.tensor_copy(
                        vg[b1][:, n2 * 384 : (n2 + 1) * 384], pm[64 : 64 + KPAD, :]
                    )

            atg = []
            for f in range(6):
                atg.append(agpool.tile([128, XG], BF16, name="atg", tag="atg"))

            # interleave plan for GEMM A (dense pass-through): 3 psum-groups
            # of A per attention batch
            a_groups = [(ct, tch) for ct in range(6) for tch in range(4)]

            def emit_a_group(ct, tch):
                sl = slice(tch * TCH, (tch + 1) * TCH)
                pm = ps_mm.tile([128, TCH], F32, tag="mm")
                for c in range(6):
                    nc.tensor.matmul(
                        pm[:], wf_sb[c][:, ct * 128 : (ct + 1) * 128], xb[c][:, sl],
                        start=(c == 0), stop=(c == 5),
                    )
                ot = opool.tile([128, TCH], F32, name="ot", tag="ot")
                nc.scalar.activation(ot[:], pm[:], AF.Identity, bias=bp_sb[ct][:])
                nc.sync.dma_start(out_d[ct * 128 : (ct + 1) * 128, sl], ot[:])

            # ---- attention on gathered tokens, batch-major
            for b in range(BL):
                gsl = slice(b * KPAD, (b + 1) * KPAD)
                for q in range(3):
                    pps = {}
                    zps = ps_z.tile([97, KPAD], F32, tag="z")
                    pvs = []
                    for k in range(4):
                        h = 4 * q + k
                        hp, hb = h // 2, (h % 2) * 64
                        ss = ps_s.tile([KPAD, KPAD], F32, tag="s")
                        nc.tensor.matmul(
                            ss[:], kg[hp][hb : hb + 64, gsl], qg[hp][hb : hb + 64, gsl]
                        )
                        pp = ppool.tile([KPAD, KPAD], BF16, tag="pp")
                        nc.scalar.activation(
                            pp[:], ss[:], AF.Exp, bias=biasg[b][0:KPAD, :]
                        )
                        pps[k] = pp
                    for pair in range(2):
                        pv2 = ps_pv.tile([128, KPAD], F32, tag="pv")
                        pvs.append(pv2)
                        for hh in range(2):
                            k = 2 * pair + hh
                            h = 4 * q + k
                            nc.tensor.matmul(
                                pv2[64 * hh : 64 * hh + 64, :],
                                vg[b][0:KPAD, h * 64 : (h + 1) * 64],
                                pps[k][:],
                            )
                            nc.tensor.matmul(
                                zps[32 * k : 32 * k + 1, :],
                                ones_bf[0:KPAD, :],
                                pps[k][:],
                                tile_position=(0, 32 * k),
                            )
                    rzq = rzpool.tile([97, KPAD], F32, tag="rz")
                    nc.vector.reciprocal(rzq[:], zps[:])
                    qq = b * 3 + q
                    nc.gpsimd.dma_start(rzd[qq, :, :], rzq[0 : 97 : 32, :])
                    for pair in range(2):
                        h0 = 4 * q + 2 * pair
                        hp = h0 // 2
                        zbb = zbpool.tile([128, KPAD], F32, tag="zbb")
                        for hh in range(2):
                            k = 2 * pair + hh
                            nc.gpsimd.dma_start(
                                zbb[64 * hh : 64 * hh + 64, :],
                                rzd[qq, k : k + 1, :].to_broadcast((64, KPAD)),
                            )
                        nc.vector.tensor_tensor(
                            atg[hp][:, gsl], pvs[pair][:], zbb[:], op=OP.mult
                        )
                # 3 dense-pass groups between batches keep the PE fed
                for gi in range(3):
                    idx = b * 3 + gi
                    if idx < len(a_groups):
                        emit_a_group(*a_groups[idx])

            for ct, tch in a_groups[24:]:
                emit_a_group(ct, tch)

            # ---- GEMM D: proj of the gathered (kept) columns
            for ct in range(6):
                pm = ps_mm.tile([128, XG], F32, tag="mm")
                for f in range(6):
                    nc.tensor.matmul(
                        pm[:], wpt_sb[f][:, ct * 128 : (ct + 1) * 128], atg[f][:],
                        start=(f == 0), stop=(f == 5),
                    )
                ot = opool.tile([128, XG], F32, name="ots", tag="ot")
                nc.scalar.activation(ot[:], pm[:], AF.Identity, bias=bp_sb[ct][:])
                nc.sync.dma_start(outs_d[ct * 128 : (ct + 1) * 128, :], ot[:])

    _split_multi_waits(nc)
    return nc


_NC = None


def _get_nc():
    global _NC
    if _NC is None:
        _NC = build_nc()
    return _NC


def _host_gate(x, g1, g2, Wm, bm):
    """ts gate on the host (tiny GEMV), returns kept index lists per batch."""
    logits = (
        x[:, 1:].reshape(-1, C).astype(np.float32) @ np.asarray(Wm, np.float32).T
    ).reshape(B, N - 1) + np.asarray(bm, np.float32)[0]
    thr = 5.0 * np.log(1.5) - np.asarray(g1, np.float32)[..., 0] + np.asarray(
        g2, np.float32
    )[..., 0]
    keep = logits > thr  # (B, N-1)
    idx = []
    for b in range(B):
        ii = [0] + [int(i) + 1 for i in np.nonzero(keep[b])[0]]
        assert len(ii) <= KPAD, f"batch {b}: {len(ii)} kept tokens > KPAD={KPAD}"
        idx.append(ii)
    return idx


def make_in_maps(x, g1, g2, Wq, Wk, Wv, Wp, bp, Wm, bm):
    import ml_dtypes

    bf16 = ml_dtypes.bfloat16
    x = np.asarray(x, np.float32)
    idx = _host_gate(x, g1, g2, Wm, bm)
    Wq, Wk, Wv, Wp = (np.asarray(w, np.float32) for w in (Wq, Wk, Wv, Wp))
    wqk = np.ascontiguousarray(np.concatenate([Wq, Wk], 0).T.astype(bf16))
    wvt = np.ascontiguousarray(Wv.T.astype(bf16))
    wf = np.ascontiguousarray((Wp @ Wv).T.astype(bf16))
    wpt = np.ascontiguousarray(Wp.T.astype(bf16))
    bpt = np.ascontiguousarray(np.asarray(bp, np.float32).reshape(6, 128))
    in_maps = []
    for i in range(NCORES):
        xs = x[i * BL : (i + 1) * BL]  # (BL, N, C)
        xtp = xs.reshape(T, C).T  # (C, T)
        cols = np.zeros(XG, np.int64)
        bgm = np.full((BL, KPAD), -NEG, np.float32)
        for bl in range(BL):
            ii = idx[i * BL + bl]
            kb = len(ii)
            cols[bl * KPAD : bl * KPAD + kb] = bl * N + np.asarray(ii)
            bgm[bl, :kb] = 0.0
        xgp = np.ascontiguousarray(xtp[:, cols].astype(bf16))
        in_maps.append(
            {
                "xb": np.ascontiguousarray(xtp.astype(bf16)),
                "xg": xgp,
                "wqk": wqk,
                "wvt": wvt,
                "wf": wf,
                "wpt": wpt,
                "bpt": bpt,
                "bgm": bgm,
            }
        )
    return in_maps, idx


def run(in_maps, trace=False, tmpdir=None):
    nc = _get_nc()
    return run_bass_kernel_spmd(
        nc, in_maps, core_ids=list(range(NCORES)), trace=trace, tmpdir=tmpdir
    )


def merge_outputs(res, idx):
    out = np.empty((B, N, C), np.float32)
    for i in range(NCORES):
        dense = res.results[i]["outt"].T.reshape(BL, N, C)
        sparse = res.results[i]["outs"].T  # (XG, C)
        out[i * BL : (i + 1) * BL] = dense
        for bl in range(BL):
            ii = idx[i * BL + bl]
            out[i * BL + bl, ii, :] = sparse[bl * KPAD : bl * KPAD + len(ii)]
    return out


def kernel(**inputs):
    in_maps, idx = make_in_maps(**inputs)
    res = run(in_maps)
    return merge_outputs(res, idx)
